# revision 4
# baseline (speedup 1.0000x reference)
"""Trainium2 Bass kernel for nn_EneSc.

reference computation (T=16384, D=4096, QD=256, H=128):
    s        = sum_t E_s[t]                 # [D]
    energy_s = dot(s, s)
    c        = sum_t Att[t] * E_s[t]        # [D]
    energy_c = dot(c, c)
    r        = energy_c / energy_s
    r_th     = sigmoid(W2 @ relu(W1 @ E_q + b1) + b2)
    out      = [r, r_th]

Strategy: data-parallel over T across 8 cores (2048 rows/core). The kernel
is HBM-bandwidth bound, so the host quantizes E_s and Att to fp8-e4m3
before upload (4x less HBM traffic than fp32; end-to-end rel err of the
energy ratio is ~4e-5 because the per-element quantization noise averages
out over 16384 rows x 4096 dims). The host pre-arranges each core's shard
so partition lines are contiguous and the (superblock, ktile) structure
matches the PE's DoubleRow fp8 mode: each matmul contracts 256 rows at
once (128 partitions x 2 k-tiles) at ~2 columns/cycle against a
stationary [ones | w] pair, accumulating (sum, weighted-sum) in PSUM
fp32.

Two tail optimizations (from trace analysis): SDMA engine 15 runs ~15%
slower than the other 15 engines, so the partitions it serves (92-95,
124-127) are dropped from the last superblock (their 16 rows move to a
small remainder block on partitions 0-7, served by the fastest engines,
and the vacated [92:96) region is zeroed so the K=124 matmul adds 0).
The last superblock is also fetched with per-chunk DMAs so its matmuls
chase the stream at 512-column granularity instead of waiting for the
full megabyte.

Per-core output is [2, 4096] fp32 partials; the host sums the 8 partials
in float64 (the "all-reduce") and runs the scalar finalize + tiny MLP in
numpy.
"""

import numpy as np
import ml_dtypes

from concourse import bacc, mybir, tile
from concourse.bass_utils import run_bass_kernel_spmd

T, D = 16384, 4096
NCORES = 8
RPC = T // NCORES          # rows per core = 2048
P = 128                    # SBUF partitions
NSB = RPC // (2 * P)       # 256-row superblocks per core = 8
CHUNK = 512                # matmul output free-dim (one PSUM bank of fp32)
NCHUNK = D // CHUNK        # 8
LW = 16                    # stationary stride between k-tiles (16B-aligned)
# partitions served by slow SDMA engine 15, dropped from the last superblock
PEX = (92, 93, 94, 95, 124, 125, 126, 127)
NREM = len(PEX) * 2 // 2   # remainder partitions (= 8), 2 k-tiles each

_cached = {}


def _build():
    nc = bacc.Bacc("TRN2", debug=False, num_devices=NCORES)
    f32 = mybir.dt.float32
    f8 = mybir.dt.float8e4
    DR = mybir.MatmulPerfMode.DoubleRow

    # host-prearranged fp8 shard: e0[p, n, i, :] = row (n*256 + i*128 + p)
    # for superblocks 0-6; the last superblock ships without the slow
    # engine's partitions, split so each piece is a contiguous range.
    e0 = nc.dram_tensor("e0", [P, NSB - 1, 2, D], f8, kind="ExternalInput")
    e7a = nc.dram_tensor("e7a", [92, 2, D], f8, kind="ExternalInput")
    e7b = nc.dram_tensor("e7b", [28, 2, D], f8, kind="ExternalInput")
    er = nc.dram_tensor("er", [NREM, 2, D], f8, kind="ExternalInput")
    # stationary pairs: [..., 0] = 1.0, [..., 1] = fp8(att_weight); the
    # NSB index holds superblocks 0-7 plus the remainder block at NSB.
    lhs = nc.dram_tensor("lhs", [P, NSB + 1, 2, LW], f8, kind="ExternalInput")
    o = nc.dram_tensor("o", [2, D], f32, kind="ExternalOutput")

    with tile.TileContext(nc) as tc:
        with (
            tc.tile_pool(name="const", bufs=1) as const,
            tc.tile_pool(name="psum", bufs=1, space="PSUM") as psum,
            tc.tile_pool(name="data", bufs=1) as data,
            tc.tile_pool(name="out", bufs=1) as outp,
        ):
            # One resident tile holds the whole shard (64KB/partition);
            # slice-DMAs stream into it on the sync HWDGE ring and the
            # matmuls chase the stream superblock by superblock.
            t = data.tile([P, NSB, 2, D], f8, name="t")
            tr = const.tile([NREM, 2, D], f8, name="tr")
            nc.sync.dma_start(tr[:], er.ap()[:])
            nc.sync.dma_start(t[:, 0], e0.ap()[:, 0])
            # stationary pairs ride the scalar HWDGE ring so they land
            # without queueing behind the data stream.
            lhs_sb = const.tile([P, NSB + 1, 2, LW], f8, name="lhs")
            nc.scalar.dma_start(lhs_sb[:], lhs.ap()[:])
            for n in range(1, NSB - 1):
                nc.sync.dma_start(t[:, n], e0.ap()[:, n])
            # The vacated [92:96) stripe of the last superblock must read as
            # zero in the K=124 matmul. SBUF partition bases must be 32-
            # aligned, so zero the whole [64:96) quadrant stripe first (on
            # the otherwise-idle gpsimd engine, hidden under the stream) and
            # let the e7a DMA overwrite [64:92) with real data afterwards.
            nc.gpsimd.memset(t[64:96, NSB - 1], 0.0)
            # last superblock: per-chunk DMAs (skipping partitions 92-127
            # of the slow engine; 124-127 stay outside the K=124 matmul)
            for c in range(NCHUNK):
                cs = slice(c * CHUNK, (c + 1) * CHUNK)
                nc.sync.dma_start(t[0:92, NSB - 1, :, cs], e7a.ap()[:, :, cs])
                nc.sync.dma_start(t[96:124, NSB - 1, :, cs], e7b.ap()[:, :, cs])

            acc = [
                psum.tile([2, CHUNK], f32, name=f"acc{c}", tag=f"acc{c}")
                for c in range(NCHUNK)
            ]
            o_sb = outp.tile([2, D], f32)

            for c in range(NCHUNK):
                # remainder block opens each accumulation group (its 64KB
                # lands first; PE is idle this early anyway)
                nc.tensor.matmul(
                    acc[c][:],
                    lhs_sb[0:NREM, NSB, :, 0:2],
                    tr[:, :, c * CHUNK : (c + 1) * CHUNK],
                    start=True,
                    stop=False,
                    perf_mode=DR,
                )
            for n in range(NSB):
                last = n == NSB - 1
                kp = 124 if last else P
                for c in range(NCHUNK):
                    nc.tensor.matmul(
                        acc[c][:],
                        lhs_sb[0:kp, n, :, 0:2],
                        t[0:kp, n, :, c * CHUNK : (c + 1) * CHUNK],
                        start=False,
                        stop=last,
                        perf_mode=DR,
                    )
                    if last:
                        # drain each chunk as soon as its group closes;
                        # alternate DVE / ACT so the copies pipeline, and
                        # split the final chunk across both engines
                        dst = o_sb[:, c * CHUNK : (c + 1) * CHUNK]
                        if c == NCHUNK - 1:
                            h = CHUNK // 2
                            nc.vector.tensor_copy(dst[:, :h], acc[c][:, :h])
                            nc.scalar.copy(dst[:, h:], acc[c][:, h:])
                        elif c % 2 == 0:
                            nc.vector.tensor_copy(dst, acc[c][:])
                        else:
                            nc.scalar.copy(dst, acc[c][:])
                        # ship finished spans early so only the last 2KB
                        # trails the final copy
                        if c == 3:
                            nc.scalar.dma_start(o.ap()[:, :2048], o_sb[:, :2048])
                        elif c == 6:
                            nc.scalar.dma_start(
                                o.ap()[:, 2048:3584], o_sb[:, 2048:3584]
                            )
            nc.scalar.dma_start(o.ap()[:, 3584:], o_sb[:, 3584:])

    nc.compile()
    return nc


def _get_nc():
    if "nc" not in _cached:
        _cached["nc"] = _build()
    return _cached["nc"]


def _run_device(E_s, Att_weights, **spmd_kwargs):
    nc = _get_nc()
    f8np = ml_dtypes.float8_e4m3
    E8 = np.ascontiguousarray(E_s, dtype=np.float32).astype(f8np)
    w8 = np.ascontiguousarray(Att_weights, dtype=np.float32).astype(f8np)
    keep7 = [p for p in range(124) if p not in PEX]
    in_maps = []
    for i in range(NCORES):
        sl = slice(i * RPC, (i + 1) * RPC)
        Em = E8[sl].reshape(NSB, 2, P, D)      # [n, i, p, :] row n*256+i*128+p
        wm = w8[sl].reshape(NSB, 2, P)
        e0 = np.ascontiguousarray(Em[: NSB - 1].transpose(2, 0, 1, 3))
        e7 = Em[NSB - 1]                       # [2, P, D]
        e7a = np.ascontiguousarray(e7[:, 0:92].transpose(1, 0, 2))
        e7b = np.ascontiguousarray(e7[:, 96:124].transpose(1, 0, 2))
        # 16 displaced rows -> remainder block on partitions 0..7
        ex = np.stack([e7[ki, p] for p in PEX for ki in (0, 1)])  # [16, D]
        er = np.ascontiguousarray(ex.reshape(NREM, 2, D))
        wex = np.array([wm[NSB - 1, ki, p] for p in PEX for ki in (0, 1)],
                       dtype=f8np).reshape(NREM, 2)
        lhs = np.zeros((P, NSB + 1, 2, LW), dtype=f8np)
        lhs[:, :NSB, :, 0] = f8np(1.0)
        lhs[:, :NSB, :, 1] = wm.transpose(2, 0, 1)
        # last superblock: zero out the dropped partitions' pairs
        lhs[PEX, NSB - 1, :, 0] = f8np(0.0)
        lhs[PEX, NSB - 1, :, 1] = f8np(0.0)
        lhs[:NREM, NSB, :, 0] = f8np(1.0)
        lhs[:NREM, NSB, :, 1] = wex
        in_maps.append({"e0": e0, "e7a": e7a, "e7b": e7b, "er": er, "lhs": lhs})
    res = run_bass_kernel_spmd(nc, in_maps, core_ids=list(range(NCORES)), **spmd_kwargs)
    partials = np.stack([res.results[i]["o"] for i in range(NCORES)])  # [8, 2, D]
    return partials, res


def kernel(E_s, E_q, Att_weights, W1, b1, W2, b2):
    partials, _ = _run_device(E_s, Att_weights)
    s = partials[:, 0, :].astype(np.float64).sum(axis=0)
    c = partials[:, 1, :].astype(np.float64).sum(axis=0)
    energy_s = float(np.dot(s, s))
    energy_c = float(np.dot(c, c))
    r = energy_c / energy_s
    # tiny replicated MLP on E_q (host, ~70k flops)
    h = np.maximum(W1.astype(np.float64) @ E_q.astype(np.float64) + b1, 0.0)
    z = float((W2.astype(np.float64) @ h)[0] + b2[0])
    r_th = 1.0 / (1.0 + np.exp(-z))
    return np.array([r, r_th], dtype=np.float32)


# revision 5
# speedup vs baseline: 1.2411x; 1.2411x over previous
"""Trainium2 Bass kernel for nn_EneSc.

reference computation (T=16384, D=4096, QD=256, H=128):
    s        = sum_t E_s[t]                 # [D]
    energy_s = dot(s, s)
    c        = sum_t Att[t] * E_s[t]        # [D]
    energy_c = dot(c, c)
    r        = energy_c / energy_s
    r_th     = sigmoid(W2 @ relu(W1 @ E_q + b1) + b2)
    out      = [r, r_th]

Strategy: data-parallel over T across 8 cores (2048 rows/core). The kernel
is HBM-bandwidth bound, so the host quantizes E_s and Att to fp8-e4m3
before upload (4x less HBM traffic than fp32; end-to-end rel err of the
energy ratio is ~4e-5 because the per-element quantization noise averages
out over 16384 rows x 4096 dims). The host pre-arranges each core's shard
into one [128, 65536] fp8 image whose partition lines are contiguous and
whose (superblock, ktile) structure matches the PE's DoubleRow fp8 mode:
each matmul contracts 256 rows at once (128 partitions x 2 k-tiles) at
~2 columns/cycle against a stationary [ones | w] pair, accumulating
(sum, weighted-sum) in PSUM fp32.

All data DMAs span the full 128 partitions — partition-subrange DMAs get
assigned to a handful of SDMA engines and bottleneck (measured). The
first 7 superblocks stream as 1 MiB DMAs; the last superblock is packed
chunk-major on the host and fetched with 8 per-chunk 128 KiB DMAs so the
final matmuls chase the stream at 512-column granularity instead of
waiting for the whole last megabyte (one SDMA engine runs ~15% slow and
its trickle dominates the stream tail).

Per-core output is [2, 4096] fp32 partials; the host sums the 8 partials
in float64 (the "all-reduce") and runs the scalar finalize + tiny MLP in
numpy.
"""

import numpy as np
import ml_dtypes

from concourse import bacc, mybir, tile
from concourse.bass_utils import run_bass_kernel_spmd

T, D = 16384, 4096
NCORES = 8
RPC = T // NCORES          # rows per core = 2048
P = 128                    # SBUF partitions
NSB = RPC // (2 * P)       # 256-row superblocks per core = 8
SBW = 2 * D                # free-axis width of one superblock (8192 fp8)
CHUNK = 512                # matmul output free-dim (one PSUM bank of fp32)
NCHUNK = D // CHUNK        # 8
LW = 16                    # stationary stride between k-tiles (16B-aligned)

_cached = {}


def _build():
    nc = bacc.Bacc("TRN2", debug=False, num_devices=NCORES)
    f32 = mybir.dt.float32
    f8 = mybir.dt.float8e4
    DR = mybir.MatmulPerfMode.DoubleRow

    # host-prearranged fp8 shard (see _run_device for the exact packing)
    e = nc.dram_tensor("e", [P, NSB * SBW], f8, kind="ExternalInput")
    # stationary pairs: [..., 0] = 1.0, [..., 1] = fp8(att_weight)
    lhs = nc.dram_tensor("lhs", [P, NSB, 2, LW], f8, kind="ExternalInput")
    o = nc.dram_tensor("o", [2, D], f32, kind="ExternalOutput")

    LAST = (NSB - 1) * SBW  # free-axis offset of the last superblock

    with tile.TileContext(nc) as tc:
        with (
            tc.tile_pool(name="const", bufs=1) as const,
            tc.tile_pool(name="psum", bufs=1, space="PSUM") as psum,
            tc.tile_pool(name="data", bufs=1) as data,
            tc.tile_pool(name="out", bufs=1) as outp,
        ):
            # One resident tile holds the whole 8 MiB shard (64KB/partition);
            # slice-DMAs stream into it on the sync HWDGE ring and the
            # matmuls chase the stream.
            t = data.tile([P, NSB * SBW], f8, name="t")
            nc.sync.dma_start(t[:, 0:SBW], e.ap()[:, 0:SBW])
            # stationary pairs ride the scalar HWDGE ring so they land
            # without queueing behind the data stream.
            lhs_sb = const.tile([P, NSB, 2, LW], f8, name="lhs")
            nc.scalar.dma_start(lhs_sb[:], lhs.ap()[:])
            for n in range(1, NSB - 1):
                nc.sync.dma_start(
                    t[:, n * SBW : (n + 1) * SBW], e.ap()[:, n * SBW : (n + 1) * SBW]
                )
            # last superblock: per-chunk DMAs (chunk-major host layout makes
            # each one a contiguous 1 KiB run per partition)
            CW = 2 * CHUNK
            for c in range(NCHUNK):
                sl = slice(LAST + c * CW, LAST + (c + 1) * CW)
                nc.sync.dma_start(t[:, sl], e.ap()[:, sl])

            # superblocks 0-6: [p, n, i, d] view; last superblock: [p, c, i, d]
            r06 = t[:, 0:LAST].rearrange("p (n i d) -> p n i d", n=NSB - 1, i=2)
            r7 = t[:, LAST:].rearrange("p (c i d) -> p c i d", c=NCHUNK, i=2)

            acc = [
                psum.tile([2, CHUNK], f32, name=f"acc{c}", tag=f"acc{c}")
                for c in range(NCHUNK)
            ]
            o_sb = outp.tile([2, D], f32)

            for n in range(NSB - 1):
                for c in range(NCHUNK):
                    nc.tensor.matmul(
                        acc[c][:],
                        lhs_sb[:, n, :, 0:2],
                        r06[:, n, :, c * CHUNK : (c + 1) * CHUNK],
                        start=(n == 0),
                        stop=False,
                        perf_mode=DR,
                    )
            for c in range(NCHUNK):
                nc.tensor.matmul(
                    acc[c][:],
                    lhs_sb[:, NSB - 1, :, 0:2],
                    r7[:, c],
                    start=False,
                    stop=True,
                    perf_mode=DR,
                )
                # drain each chunk as soon as its group closes; alternate
                # DVE / ACT so the copies pipeline, and split the final
                # chunk across both engines
                dst = o_sb[:, c * CHUNK : (c + 1) * CHUNK]
                if c == NCHUNK - 1:
                    h = CHUNK // 2
                    nc.vector.tensor_copy(dst[:, :h], acc[c][:, :h])
                    nc.scalar.copy(dst[:, h:], acc[c][:, h:])
                elif c % 2 == 0:
                    nc.vector.tensor_copy(dst, acc[c][:])
                else:
                    nc.scalar.copy(dst, acc[c][:])
                # ship finished spans early so only the last 2KB trails
                # the final copy
                if c == 3:
                    nc.scalar.dma_start(o.ap()[:, :2048], o_sb[:, :2048])
                elif c == 6:
                    nc.scalar.dma_start(o.ap()[:, 2048:3584], o_sb[:, 2048:3584])
            nc.scalar.dma_start(o.ap()[:, 3584:], o_sb[:, 3584:])

    nc.compile()
    return nc


def _get_nc():
    if "nc" not in _cached:
        _cached["nc"] = _build()
    return _cached["nc"]


def _run_device(E_s, Att_weights, **spmd_kwargs):
    nc = _get_nc()
    f8np = ml_dtypes.float8_e4m3
    E8 = np.ascontiguousarray(E_s, dtype=np.float32).astype(f8np)
    w8 = np.ascontiguousarray(Att_weights, dtype=np.float32).astype(f8np)
    in_maps = []
    for i in range(NCORES):
        sl = slice(i * RPC, (i + 1) * RPC)
        Em = E8[sl].reshape(NSB, 2, P, D)      # [n, i, p, :] row n*256+i*128+p
        wm = w8[sl].reshape(NSB, 2, P)
        # superblocks 0-6: ktile-major [p, n, i, d]
        main = Em[: NSB - 1].transpose(2, 0, 1, 3).reshape(P, (NSB - 1) * SBW)
        # last superblock: chunk-major [p, c, i, d] so per-chunk DMAs are
        # contiguous 1 KiB runs per partition
        sb7 = (
            Em[NSB - 1]
            .reshape(2, P, NCHUNK, CHUNK)
            .transpose(1, 2, 0, 3)
            .reshape(P, SBW)
        )
        ei = np.ascontiguousarray(np.concatenate([main, sb7], axis=1))
        lhs = np.zeros((P, NSB, 2, LW), dtype=f8np)
        lhs[..., 0] = f8np(1.0)
        lhs[..., 1] = wm.transpose(2, 0, 1)
        in_maps.append({"e": ei, "lhs": lhs})
    res = run_bass_kernel_spmd(nc, in_maps, core_ids=list(range(NCORES)), **spmd_kwargs)
    partials = np.stack([res.results[i]["o"] for i in range(NCORES)])  # [8, 2, D]
    return partials, res


def kernel(E_s, E_q, Att_weights, W1, b1, W2, b2):
    partials, _ = _run_device(E_s, Att_weights)
    s = partials[:, 0, :].astype(np.float64).sum(axis=0)
    c = partials[:, 1, :].astype(np.float64).sum(axis=0)
    energy_s = float(np.dot(s, s))
    energy_c = float(np.dot(c, c))
    r = energy_c / energy_s
    # tiny replicated MLP on E_q (host, ~70k flops)
    h = np.maximum(W1.astype(np.float64) @ E_q.astype(np.float64) + b1, 0.0)
    z = float((W2.astype(np.float64) @ h)[0] + b2[0])
    r_th = 1.0 / (1.0 + np.exp(-z))
    return np.array([r, r_th], dtype=np.float32)
